# revision 1
# baseline (speedup 1.0000x reference)
"""APPNP (K-step propagation over an MLP) on 8 TRN2 NeuronCores.

Strategy:
  - Nodes are partitioned contiguously across the 8 cores (12500 per core,
    padded to 12544 = 98*128 device-side slots). Within each core, nodes are
    RELABELED in descending in-degree order so that fixed-degree padding per
    128-node block is cheap.
  - The dense MLP (lin1/lin2) is data-parallel over nodes.
  - Propagation uses A_hat h = dinv * (A g + g), g = dinv * h, so there is no
    per-edge scaling. Each step: AllGather of the g shard into a full table,
    then a fixed-degree pull per 128-node block: one indirect DMA per slot
    column fetches g[src] for the block's 128 nodes (one 256B row per
    partition). The block's slot columns are reduced on the vector engine
    into agg; a small per-node epilogue forms the next g.
  - log_softmax at the end on the local shard; host un-permutes rows.
"""

import sys

sys.path.insert(0, "/opt/trn_rl_repo")

import numpy as np

NCORES = 8
ALPHA = 0.1
F_IN = 512
HID = 64
C = 40
CP = 64  # padded class dim; table rows are CP floats = 256B


class Cfg:
    def __init__(self, N, LV, L, K):
        assert N == NCORES * LV
        assert L % 128 == 0 and LV < L
        self.N = N
        self.LV = LV
        self.L = L
        self.K = K
        self.T = L // 128
        self.NPAD = NCORES * L


FULL = Cfg(N=100000, LV=12500, L=12544, K=10)


def preprocess(cfg, x, edge_index, w1, b1, w2, b2):
    N, LV, L, T = cfg.N, cfg.LV, cfg.L, cfg.T
    src = np.asarray(edge_index[0], dtype=np.int64)
    dst = np.asarray(edge_index[1], dtype=np.int64)

    deg_in = np.bincount(dst, minlength=N)
    deg = (deg_in + 1).astype(np.float32)
    dinv = (1.0 / np.sqrt(deg)).astype(np.float32)

    # per-core relabeling: slot s of core r holds node ord_g[r][s] (global id)
    ord_g = np.empty((NCORES, LV), dtype=np.int64)
    slot_of = np.empty(N, dtype=np.int64)
    for r in range(NCORES):
        ids = np.arange(r * LV, (r + 1) * LV, dtype=np.int64)
        o = ids[np.argsort(-deg_in[ids], kind="stable")]
        ord_g[r] = o
        slot_of[o] = np.arange(LV, dtype=np.int64)

    # table row of a global node in the relabeled padded table
    trow = (np.arange(N) // LV) * L + slot_of

    # per-block slot counts (cross-core max)
    Wb = np.zeros(T, dtype=np.int64)
    for r in range(NCORES):
        d_slots = np.zeros(L, dtype=np.int64)
        d_slots[:LV] = deg_in[ord_g[r]]
        Wb = np.maximum(Wb, d_slots.reshape(T, 128).max(axis=1))
    Wb = np.maximum(Wb, 1)
    W = [int(v) for v in Wb]
    coff = np.concatenate([[0], np.cumsum(Wb)]).astype(np.int64)
    NSLOT = int(coff[-1])
    WMAX = int(Wb.max())

    # edges sorted by dst for per-node in-edge lists
    order_e = np.argsort(dst, kind="stable")
    dst_s = dst[order_e]
    src_s = src[order_e]
    starts = np.zeros(N + 1, dtype=np.int64)
    starts[1:] = np.cumsum(deg_in)

    w2p = np.zeros((HID, CP), dtype=np.float32)
    w2p[:, :C] = np.asarray(w2, dtype=np.float32)
    b2p = np.zeros((CP,), dtype=np.float32)
    b2p[:C] = np.asarray(b2, dtype=np.float32)
    w1s = np.asarray(w1, dtype=np.float32).reshape(4, 128, HID).transpose(1, 0, 2).copy()
    b1r = np.tile(np.asarray(b1, dtype=np.float32), (128, 1))
    b2r = np.tile(b2p, (128, 1))
    ident = np.eye(128, dtype=np.float32)

    xf = np.asarray(x, dtype=np.float32)
    in_maps = []
    for r in range(NCORES):
        ZR = r * L + LV  # guaranteed-zero table row of our own rank
        pad = np.full((L, WMAX), ZR, dtype=np.int64)
        lo, hi = np.searchsorted(dst_s, [r * LV, (r + 1) * LV])
        dsts_r = dst_s[lo:hi]
        srcs_r = src_s[lo:hi]
        pos_r = np.arange(lo, hi) - starts[dsts_r]
        rows_r = slot_of[dsts_r]
        pad[rows_r, pos_r] = trow[srcs_r]

        idx32 = np.full((128, NSLOT), ZR, dtype=np.int32)
        for t in range(T):
            w = W[t]
            idx32[:, coff[t] : coff[t] + w] = pad[t * 128 : (t + 1) * 128, :w]

        xs = np.zeros((L, F_IN), dtype=np.float32)
        xs[:LV] = xf[ord_g[r]]
        xt = np.ascontiguousarray(xs.T)

        dl = np.zeros((L,), dtype=np.float32)
        dl[:LV] = dinv[ord_g[r]]
        dpt = np.ascontiguousarray(dl.reshape(T, 128).T)

        in_maps.append(
            {
                "xt": xt,
                "w1s": w1s,
                "b1r": b1r,
                "w2p": w2p,
                "b2r": b2r,
                "ident": ident,
                "dinv": dpt,
                "d1a": ((1.0 - ALPHA) * dpt).astype(np.float32),
                "d2a": ((1.0 - ALPHA) * dpt * dpt).astype(np.float32),
                "gidx": idx32,
            }
        )
    meta = {
        "W": W,
        "coff": [int(v) for v in coff],
        "NSLOT": NSLOT,
        "WMAX": WMAX,
        "ord_g": ord_g,
    }
    return in_maps, meta


def build(cfg, meta):
    import concourse.bacc as bacc
    import concourse.bass as bass
    import concourse.mybir as mybir
    import concourse.tile as tile

    fp32 = mybir.dt.float32
    bf16 = mybir.dt.bfloat16
    i32 = mybir.dt.int32
    AF = mybir.ActivationFunctionType
    ALU = mybir.AluOpType

    L, T, K, NPAD = cfg.L, cfg.T, cfg.K, cfg.NPAD
    W, coff, NSLOT, WMAX = meta["W"], meta["coff"], meta["NSLOT"], meta["WMAX"]

    nc = bacc.Bacc("TRN2", target_bir_lowering=False, debug=False, num_devices=NCORES)

    xt_e = nc.declare_dram_parameter("xt", [F_IN, L], fp32, isOutput=False)
    w1s_e = nc.declare_dram_parameter("w1s", [128, 4, HID], fp32, isOutput=False)
    b1r_e = nc.declare_dram_parameter("b1r", [128, HID], fp32, isOutput=False)
    w2p_e = nc.declare_dram_parameter("w2p", [HID, CP], fp32, isOutput=False)
    b2r_e = nc.declare_dram_parameter("b2r", [128, CP], fp32, isOutput=False)
    ident_e = nc.declare_dram_parameter("ident", [128, 128], fp32, isOutput=False)
    dinv_e = nc.declare_dram_parameter("dinv", [128, T], fp32, isOutput=False)
    d1a_e = nc.declare_dram_parameter("d1a", [128, T], fp32, isOutput=False)
    d2a_e = nc.declare_dram_parameter("d2a", [128, T], fp32, isOutput=False)
    gidx_e = nc.declare_dram_parameter("gidx", [128, NSLOT], i32, isOutput=False)
    out_e = nc.declare_dram_parameter("out", [L, C], fp32, isOutput=True)

    with tile.TileContext(nc) as tc:
        with (
            tc.tile_pool(name="res", bufs=1) as res,
            tc.tile_pool(name="dram", bufs=1, space="DRAM") as dram,
            tc.tile_pool(name="mlp", bufs=3) as mlp,
            tc.tile_pool(name="mpsum", bufs=2, space="PSUM") as mpsum,
            tc.tile_pool(name="lp", bufs=4) as lp,
        ):
            g_cur = res.tile([128, T, CP], fp32)
            a_g0 = res.tile([128, T, C], bf16)
            a_h0 = res.tile([128, T, C], bf16)
            it = res.tile([128, NSLOT], i32)
            w1_sb = res.tile([128, 4, HID], fp32)
            b1_sb = res.tile([128, HID], fp32)
            w2_sb = res.tile([HID, CP], fp32)
            b2_sb = res.tile([128, CP], fp32)
            id_sb = res.tile([128, 128], fp32)
            dinv_sb = res.tile([128, T], fp32)
            d1a_sb = res.tile([128, T], fp32)
            d2a_sb = res.tile([128, T], fp32)

            nc.sync.dma_start(out=it[:], in_=gidx_e[:, :])
            nc.sync.dma_start(out=w1_sb[:], in_=w1s_e[:, :, :])
            nc.sync.dma_start(out=b1_sb[:], in_=b1r_e[:, :])
            nc.sync.dma_start(out=w2_sb[:], in_=w2p_e[:, :])
            nc.sync.dma_start(out=b2_sb[:], in_=b2r_e[:, :])
            nc.sync.dma_start(out=id_sb[:], in_=ident_e[:, :])
            nc.sync.dma_start(out=dinv_sb[:], in_=dinv_e[:, :])
            nc.sync.dma_start(out=d1a_sb[:], in_=d1a_e[:, :])
            nc.sync.dma_start(out=d2a_sb[:], in_=d2a_e[:, :])

            xt_r = xt_e.ap().rearrange("(kb p) n -> p kb n", p=128)

            # ---- MLP
            for t in range(T):
                xk = mlp.tile([128, 4, 128], fp32, tag="xk")
                nc.sync.dma_start(out=xk[:], in_=xt_r[:, :, t * 128 : (t + 1) * 128])
                ps1 = mpsum.tile([128, HID], fp32, tag="ps1")
                for k in range(4):
                    nc.tensor.matmul(
                        ps1[:], xk[:, k, :], w1_sb[:, k, :],
                        start=(k == 0), stop=(k == 3),
                    )
                h1 = mlp.tile([128, HID], fp32, tag="h1")
                nc.vector.tensor_tensor(h1[:], ps1[:], b1_sb[:], op=ALU.add)
                nc.scalar.activation(h1[:], h1[:], AF.Relu)
                pst = mpsum.tile([128, 128], fp32, tag="pst")
                nc.tensor.transpose(pst[:HID, :], h1[:], id_sb[:])
                h1t = mlp.tile([HID, 128], fp32, tag="h1t")
                nc.vector.tensor_copy(h1t[:], pst[:HID, :])
                ps2 = mpsum.tile([128, CP], fp32, tag="ps2")
                nc.tensor.matmul(ps2[:], h1t[:], w2_sb[:], start=True, stop=True)
                h0t = mlp.tile([128, CP], fp32, tag="h0t")
                nc.vector.tensor_tensor(h0t[:], ps2[:], b2_sb[:], op=ALU.add)
                with nc.allow_low_precision(reason="alpha anchors stored bf16"):
                    nc.vector.tensor_scalar_mul(a_h0[:, t, :], h0t[:, 0:C], ALPHA)
                nc.vector.tensor_scalar(
                    g_cur[:, t, :], h0t[:], dinv_sb[:, t : t + 1], None, op0=ALU.mult
                )
                with nc.allow_low_precision(reason="alpha anchors stored bf16"):
                    nc.vector.tensor_scalar_mul(a_g0[:, t, :], g_cur[:, t, 0:C], ALPHA)

            rg = [list(range(NCORES))]
            for step in range(1, K + 1):
                ag_in = dram.tile([L, CP], fp32, name=f"agi{step}", tag=f"agi{step}")
                ag_out = dram.tile(
                    [NPAD, CP], fp32, addr_space="Shared",
                    name=f"ago{step}", tag=f"ago{step}",
                )
                ag_in_r = ag_in[:].rearrange("(t p) c -> p t c", p=128)
                nc.sync.dma_start(out=ag_in_r, in_=g_cur[:])
                nc.gpsimd.collective_compute(
                    "AllGather",
                    mybir.AluOpType.bypass,
                    replica_groups=rg,
                    ins=[ag_in.opt()],
                    outs=[ag_out.opt()],
                )

                last = step == K
                dsc = d1a_sb if last else d2a_sb
                anchor = a_h0 if last else a_g0
                for t in range(T):
                    wt = W[t]
                    gt = lp.tile([128, WMAX, CP], fp32, tag="gt")
                    for c in range(wt):
                        nc.gpsimd.indirect_dma_start(
                            out=gt[:, c, :],
                            out_offset=None,
                            in_=ag_out[:],
                            in_offset=bass.IndirectOffsetOnAxis(
                                ap=it[:, coff[t] + c : coff[t] + c + 1], axis=0
                            ),
                        )
                    agg = mlp.tile([128, C], fp32, tag="agg")
                    if wt == 1:
                        nc.vector.tensor_copy(agg[:], gt[:, 0, 0:C])
                    else:
                        gre = bass.AP(
                            gt[:].tensor,
                            gt[:].offset,
                            [gt[:].ap[0], [1, C], [CP, wt]],
                        )
                        nc.vector.tensor_reduce(
                            agg[:], gre, axis=mybir.AxisListType.X, op=ALU.add
                        )
                    nc.vector.tensor_tensor(
                        agg[:], agg[:], g_cur[:, t, 0:C], op=ALU.add
                    )
                    nc.vector.scalar_tensor_tensor(
                        g_cur[:, t, 0:C],
                        agg[:],
                        dsc[:, t : t + 1],
                        anchor[:, t, :],
                        op0=ALU.mult,
                        op1=ALU.add,
                    )

            # ---- log_softmax over first C cols of g_cur (== h_K)
            red = res.tile([128, T, 2], fp32)
            ex = res.tile([128, T, C], fp32)
            nc.vector.tensor_reduce(
                red[:, :, 0:1], g_cur[:, :, 0:C], axis=mybir.AxisListType.X, op=ALU.max
            )
            for t in range(T):
                nc.vector.tensor_scalar(
                    ex[:, t, :], g_cur[:, t, 0:C], red[:, t, 0:1], None,
                    op0=ALU.subtract,
                )
            nc.scalar.activation(ex[:], ex[:], AF.Exp)
            nc.vector.tensor_reduce(
                red[:, :, 1:2], ex[:], axis=mybir.AxisListType.X, op=ALU.add
            )
            nc.scalar.activation(red[:, :, 1:2], red[:, :, 1:2], AF.Ln)
            outt = res.tile([128, T, C], fp32)
            for t in range(T):
                nc.vector.tensor_scalar(
                    outt[:, t, :], g_cur[:, t, 0:C], red[:, t, 0:1], red[:, t, 1:2],
                    op0=ALU.subtract, op1=ALU.subtract,
                )
            out_r = out_e.ap().rearrange("(t p) c -> p t c", p=128)
            nc.sync.dma_start(out=out_r, in_=outt[:])

    nc.finalize()
    return nc


def _run(cfg, x, edge_index, w1, b1, w2, b2, trace=False):
    from concourse import bass_utils

    in_maps, meta = preprocess(cfg, x, edge_index, w1, b1, w2, b2)
    nc = build(cfg, meta)
    res = bass_utils.run_bass_kernel_spmd(
        nc, in_maps, core_ids=list(range(NCORES)), trace=trace
    )
    out = np.empty((cfg.N, C), dtype=np.float32)
    for r in range(NCORES):
        out[meta["ord_g"][r]] = res.results[r]["out"][: cfg.LV]
    return out, res


def kernel(x, edge_index, w1, b1, w2, b2):
    out, _ = _run(
        FULL,
        np.asarray(x),
        np.asarray(edge_index),
        np.asarray(w1),
        np.asarray(b1),
        np.asarray(w2),
        np.asarray(b2),
    )
    return out



# revision 11
# speedup vs baseline: 1.4879x; 1.4879x over previous
"""APPNP (K-step propagation over an MLP) on 8 TRN2 NeuronCores.

Strategy:
  - Nodes are partitioned contiguously across the 8 cores (12500 per core,
    padded to 12544 = 98*128 device-side slots). Within each core, nodes are
    RELABELED in descending in-degree order so fixed-degree padding per
    128-node block is cheap.
  - The dense MLP (lin1/lin2) is data-parallel over nodes.
  - Propagation uses A_hat h = dinv * (A g + g), g = dinv * h, so there is no
    per-edge scaling. Each step: AllGather of the fp16 g shard (padded to
    256B-strided rows) into a full table [NPAD, 128], then the per-edge
    gather runs on the DMAGather Q7 ucode (InstDMAGatherAnt): int16 indices
    limited to 32768 rows force the table into ceil(NPAD/32768) windows; per
    128-node block and window, a fixed number of slot columns W[t][q] is
    gathered with ONE dma_gather each (80B payload per edge, 256B row
    stride). Gathers round-robin over 4 SWDGE queues whose descriptor
    generation runs on dedicated Q7 core pairs in parallel. Per block, the
    vector engine reduces the gathered slots into agg (fp32), adds the self
    term, and a fused scalar_tensor_tensor epilogue forms the next g (fp16).
  - log_softmax at the end on the local shard; host un-permutes rows.
"""

import sys

sys.path.insert(0, "/opt/trn_rl_repo")

import numpy as np

NCORES = 8
ALPHA = 0.1
F_IN = 512
HID = 64
C = 40
STRIDE = 128  # fp16 elems per table row = 256B (dma_gather stride granule)
WINROWS = 32768  # max rows addressable by an int16 gather index
# 4 SWDGE queues + HWDGE + collective rings exceed the DMA ring budget and
# deadlock queues >= 1 whenever a collective is present; 3 is the max safe.
NQUEUES = 3
IDX_CHUNK_BLOCKS = 8  # blocks per idx-tile DMA


class Cfg:
    def __init__(self, N, LV, L, K):
        assert N == NCORES * LV
        assert L % 128 == 0 and LV < L
        self.N = N
        self.LV = LV
        self.L = L
        self.K = K
        self.T = L // 128
        self.NPAD = NCORES * L


FULL = Cfg(N=100000, LV=12500, L=12544, K=10)


def preprocess(cfg, x, edge_index, w1, b1, w2, b2):
    N, LV, L, T = cfg.N, cfg.LV, cfg.L, cfg.T
    NPAD = cfg.NPAD
    src = np.asarray(edge_index[0], dtype=np.int64)
    dst = np.asarray(edge_index[1], dtype=np.int64)
    E = src.shape[0]

    deg_in = np.bincount(dst, minlength=N)
    deg = (deg_in + 1).astype(np.float32)
    dinv = (1.0 / np.sqrt(deg)).astype(np.float32)

    # per-core relabeling: slot s of core r holds node ord_g[r][s] (global id)
    ord_g = np.empty((NCORES, LV), dtype=np.int64)
    slot_of = np.empty(N, dtype=np.int64)
    for r in range(NCORES):
        ids = np.arange(r * LV, (r + 1) * LV, dtype=np.int64)
        o = ids[np.argsort(-deg_in[ids], kind="stable")]
        ord_g[r] = o
        slot_of[o] = np.arange(LV, dtype=np.int64)

    # table row of a global node in the relabeled padded table
    trow = (np.arange(N) // LV) * L + slot_of

    # index windows (int16 gather index < 32768)
    WINB = list(range(0, NPAD, WINROWS)) + [NPAD]
    NQ = len(WINB) - 1
    # a guaranteed-zero table row inside each window (local index)
    zr_local = []
    pad_rows = [r * L + LV for r in range(NCORES)]
    for q in range(NQ):
        z = next(g for g in pad_rows if WINB[q] <= g < WINB[q + 1])
        zr_local.append(z - WINB[q])

    # per-edge source window + local index
    tsrc = trow[src]
    qe = tsrc // WINROWS
    le = tsrc - qe * WINROWS

    # sort edges by (dst, window); position within each (dst, window) group
    grp = dst * NQ + qe
    order = np.argsort(grp, kind="stable")
    grp_s = grp[order]
    dst_s = dst[order]
    qs_s = qe[order]
    ls_s = le[order]
    cnt = np.bincount(grp_s, minlength=N * NQ)
    starts = np.zeros(N * NQ + 1, dtype=np.int64)
    starts[1:] = np.cumsum(cnt)
    w_s = np.arange(E, dtype=np.int64) - starts[grp_s]
    cnt = cnt.reshape(N, NQ)

    # per-block per-window slot counts (cross-core max)
    W = np.zeros((T, NQ), dtype=np.int64)
    for r in range(NCORES):
        cs = np.zeros((L, NQ), dtype=np.int64)
        cs[:LV] = cnt[ord_g[r]]
        W = np.maximum(W, cs.reshape(T, 128, NQ).max(axis=1))
    wsum = W.sum(axis=1)
    WSUM_MAX = int(max(wsum.max(), 1))
    qoff = np.zeros((T, NQ), dtype=np.int64)
    qoff[:, 1:] = np.cumsum(W, axis=1)[:, :-1]
    icols = wsum * 8  # int16 idx columns per block (128 idx / 16 partitions)
    ioff = np.zeros(T + 1, dtype=np.int64)
    ioff[1:] = np.cumsum(icols)
    ICOLS = int(ioff[-1])
    coloff = ioff[:T, None] + qoff * 8  # [T, NQ]

    # idx-tile DMA chunks (groups of blocks)
    chunks = []
    t0 = 0
    while t0 < T:
        t1 = min(t0 + IDX_CHUNK_BLOCKS, T)
        chunks.append((t0, t1, int(ioff[t0]), int(ioff[t1])))
        t0 = t1
    ICH_MAX = max(c1 - c0 for _, _, c0, c1 in chunks)

    w2p = np.zeros((HID, C), dtype=np.float32)
    w2p[:, :] = np.asarray(w2, dtype=np.float32)
    b2p = np.asarray(b2, dtype=np.float32)
    w1s = np.asarray(w1, dtype=np.float32).reshape(4, 128, HID).transpose(1, 0, 2).copy()
    b1r = np.tile(np.asarray(b1, dtype=np.float32), (128, 1))
    b2r = np.tile(b2p, (128, 1))
    ident = np.eye(128, dtype=np.float32)

    xf = np.asarray(x, dtype=np.float32)
    in_maps = []
    for r in range(NCORES):
        lo, hi = np.searchsorted(dst_s, [r * LV, (r + 1) * LV])
        rows_r = slot_of[dst_s[lo:hi]]
        q_r = qs_s[lo:hi]
        w_r = w_s[lo:hi]
        l_r = ls_s[lo:hi]
        t_r = rows_r // 128
        p_r = rows_r % 128

        idx16 = np.empty((16, ICOLS), dtype=np.int16)
        for t in range(T):
            for q in range(NQ):
                a = coloff[t, q]
                idx16[:, a : a + W[t, q] * 8] = zr_local[q]
        col = coloff[t_r, q_r] + w_r * 8 + p_r // 16
        idx16[p_r % 16, col] = l_r.astype(np.int16)
        idxfull = np.tile(idx16, (8, 1))

        xs = np.zeros((L, F_IN), dtype=np.float32)
        xs[:LV] = xf[ord_g[r]]
        xt = np.ascontiguousarray(xs.T)

        dl = np.zeros((L,), dtype=np.float32)
        dl[:LV] = dinv[ord_g[r]]
        dpt = np.ascontiguousarray(dl.reshape(T, 128).T)

        in_maps.append(
            {
                "xt": xt,
                "w1s": w1s,
                "b1r": b1r,
                "w2p": w2p,
                "b2r": b2r,
                "ident": ident,
                "dinv": dpt,
                "d1a": ((1.0 - ALPHA) * dpt).astype(np.float32),
                "d2a": ((1.0 - ALPHA) * dpt * dpt).astype(np.float32),
                "gidx": idxfull,
            }
        )
    meta = {
        "W": W,
        "qoff": qoff,
        "wsum": [int(v) for v in wsum],
        "WSUM_MAX": WSUM_MAX,
        "coloff": coloff,
        "ioff": [int(v) for v in ioff],
        "ICOLS": ICOLS,
        "ICH_MAX": ICH_MAX,
        "chunks": chunks,
        "WINB": WINB,
        "NQ": NQ,
        "ord_g": ord_g,
    }
    return in_maps, meta


def _emit_dma_gather(gp, mybir, ap_utils, out_ap, in_ap, idxs_ap, num_idxs, elem_size,
                     elem_step, queue_num):
    """bass.dma_gather minus the %256 elem_size assert: the DMAGather ucode in
    non-transpose HBM-source mode only requires the row STRIDE to be a 256B
    multiple; the moved payload (elem_size) is arbitrary."""
    gp._assert_queue_num(queue_num)
    assert idxs_ap.dtype == mybir.dt.int16
    assert in_ap.dtype == out_ap.dtype
    assert ap_utils.ap_is_contiguous(out_ap.ap[1:])
    assert ap_utils.ap_is_contiguous(idxs_ap.ap[1:])
    assert in_ap.ap[-1][1] == out_ap.ap[-1][1] == elem_size
    assert out_ap.ap[0][1] * out_ap.ap[1][1] == ((num_idxs + 127) // 128) * 128
    assert in_ap.ap[0][0] == elem_step
    stride_bytes = elem_step * mybir.dt.size(in_ap.dtype)
    assert stride_bytes % 256 == 0 and 0 < stride_bytes // 256 < 256
    _in_ap = gp.lower_ap_dma(in_ap, for_custom_bir_dma=True)
    _idxs_ap = gp.lower_ap(idxs_ap)
    _out_ap = gp.lower_ap(out_ap)
    return gp.add_instruction(
        mybir.InstDMAGatherAnt(
            name=gp.bass.get_next_instruction_name(),
            ins=[*_in_ap, _idxs_ap, gp.lower_val_access(gp.to_reg(num_idxs))],
            outs=[_out_ap],
            transpose=False,
            num_idxs=num_idxs,
            elem_size=elem_size,
            stride_bytes_256=stride_bytes // 256,
            gen_mode=0,
            single_packet=True,
            queue_num=queue_num,
            sbuf_tokens_per_rank=0,
            sbuf_free_dim_per_rank=0,
            sbuf_free_dim_pad_per_rank=0,
            sbuf_byte_offset=0,
        )
    )


def build(cfg, meta):
    import concourse.bacc as bacc
    import concourse.bass as bass
    import concourse.mybir as mybir
    import concourse.tile as tile
    from concourse import ap_utils, library_config

    fp32 = mybir.dt.float32
    fp16 = mybir.dt.float16
    i16 = mybir.dt.int16
    AF = mybir.ActivationFunctionType
    ALU = mybir.AluOpType

    L, T, K, NPAD = cfg.L, cfg.T, cfg.K, cfg.NPAD
    W, qoff, wsum = meta["W"], meta["qoff"], meta["wsum"]
    WSUM_MAX, coloff, ICOLS = meta["WSUM_MAX"], meta["coloff"], meta["ICOLS"]
    ICH_MAX, chunks, WINB, NQ = meta["ICH_MAX"], meta["chunks"], meta["WINB"], meta["NQ"]

    nc = bacc.Bacc(
        "TRN2", target_bir_lowering=False, debug=False,
        num_devices=NCORES, num_swdge_queues=NQUEUES,
    )

    xt_e = nc.declare_dram_parameter("xt", [F_IN, L], fp32, isOutput=False)
    w1s_e = nc.declare_dram_parameter("w1s", [128, 4, HID], fp32, isOutput=False)
    b1r_e = nc.declare_dram_parameter("b1r", [128, HID], fp32, isOutput=False)
    w2p_e = nc.declare_dram_parameter("w2p", [HID, C], fp32, isOutput=False)
    b2r_e = nc.declare_dram_parameter("b2r", [128, C], fp32, isOutput=False)
    ident_e = nc.declare_dram_parameter("ident", [128, 128], fp32, isOutput=False)
    dinv_e = nc.declare_dram_parameter("dinv", [128, T], fp32, isOutput=False)
    d1a_e = nc.declare_dram_parameter("d1a", [128, T], fp32, isOutput=False)
    d2a_e = nc.declare_dram_parameter("d2a", [128, T], fp32, isOutput=False)
    gidx_e = nc.declare_dram_parameter("gidx", [128, ICOLS], i16, isOutput=False)
    out_e = nc.declare_dram_parameter("out", [L, C], fp32, isOutput=True)

    with tile.TileContext(nc) as tc:
        with (
            tc.tile_pool(name="res", bufs=1) as res,
            tc.tile_pool(name="dram", bufs=1, space="DRAM") as dram,
            tc.tile_pool(name="mlp", bufs=3) as mlp,
            tc.tile_pool(name="mpsum", bufs=2, space="PSUM") as mpsum,
            tc.tile_pool(name="lp", bufs=3) as lp,
            tc.tile_pool(name="ip", bufs=3) as ip,
        ):
            nc.gpsimd.load_library(library_config.mlp)

            g16 = res.tile([128, T, STRIDE], fp16)
            a_g0 = res.tile([128, T, C], fp32)
            a_h0 = res.tile([128, T, C], fp32)
            hK = res.tile([128, T, C], fp32)
            w1_sb = res.tile([128, 4, HID], fp32)
            b1_sb = res.tile([128, HID], fp32)
            w2_sb = res.tile([HID, C], fp32)
            b2_sb = res.tile([128, C], fp32)
            id_sb = res.tile([128, 128], fp32)
            dinv_sb = res.tile([128, T], fp32)
            d1a_sb = res.tile([128, T], fp32)
            d2a_sb = res.tile([128, T], fp32)

            nc.vector.memset(g16[:], 0.0)
            nc.vector.memset(hK[:], 0.0)
            nc.sync.dma_start(out=w1_sb[:], in_=w1s_e[:, :, :])
            nc.sync.dma_start(out=b1_sb[:], in_=b1r_e[:, :])
            nc.sync.dma_start(out=w2_sb[:], in_=w2p_e[:, :])
            nc.sync.dma_start(out=b2_sb[:], in_=b2r_e[:, :])
            nc.sync.dma_start(out=id_sb[:], in_=ident_e[:, :])
            nc.sync.dma_start(out=dinv_sb[:], in_=dinv_e[:, :])
            nc.sync.dma_start(out=d1a_sb[:], in_=d1a_e[:, :])
            nc.sync.dma_start(out=d2a_sb[:], in_=d2a_e[:, :])

            xt_r = xt_e.ap().rearrange("(kb p) n -> p kb n", p=128)

            # ---- MLP
            for t in range(T):
                xk = mlp.tile([128, 4, 128], fp32, tag="xk")
                nc.sync.dma_start(out=xk[:], in_=xt_r[:, :, t * 128 : (t + 1) * 128])
                ps1 = mpsum.tile([128, HID], fp32, tag="ps1")
                for k in range(4):
                    nc.tensor.matmul(
                        ps1[:], xk[:, k, :], w1_sb[:, k, :],
                        start=(k == 0), stop=(k == 3),
                    )
                h1 = mlp.tile([128, HID], fp32, tag="h1")
                nc.vector.tensor_tensor(h1[:], ps1[:], b1_sb[:], op=ALU.add)
                nc.scalar.activation(h1[:], h1[:], AF.Relu)
                pst = mpsum.tile([128, 128], fp32, tag="pst")
                nc.tensor.transpose(pst[:HID, :], h1[:], id_sb[:])
                h1t = mlp.tile([HID, 128], fp32, tag="h1t")
                nc.vector.tensor_copy(h1t[:], pst[:HID, :])
                ps2 = mpsum.tile([128, C], fp32, tag="ps2")
                nc.tensor.matmul(ps2[:], h1t[:], w2_sb[:], start=True, stop=True)
                h0t = mlp.tile([128, C], fp32, tag="h0t")
                nc.vector.tensor_tensor(h0t[:], ps2[:], b2_sb[:], op=ALU.add)
                with nc.allow_low_precision(reason="fp16 propagation table"):
                    nc.vector.tensor_scalar(
                        g16[:, t, 0:C], h0t[:], dinv_sb[:, t : t + 1], None,
                        op0=ALU.mult,
                    )
                nc.vector.tensor_scalar(
                    a_g0[:, t, :], h0t[:], dinv_sb[:, t : t + 1], ALPHA,
                    op0=ALU.mult, op1=ALU.mult,
                )
                nc.vector.tensor_scalar_mul(a_h0[:, t, :], h0t[:], ALPHA)

            rg = [list(range(NCORES))]
            # The tile framework rotates Pool-engine DMAs over
            # NUM_SWDGE_GLOBAL_SEMS=8 DMASW semaphore lanes in program order,
            # and the runtime locks each lane to one SWDGE queue. Gathers are
            # the only Pool DMAs here, so a fixed lane->queue map keeps every
            # semaphore lane on a single queue while using all 3 queues.
            QMAP = [0, 1, 2, 0, 1, 2, 0, 1]
            qrr = 0  # gather counter (== DMASW lane rotation)
            for step in range(1, K + 1):
                ag_in = dram.tile([L, STRIDE], fp16, name=f"agi{step}", tag=f"agi{step}")
                ag_out = dram.tile(
                    [NPAD, STRIDE], fp16, addr_space="Shared",
                    name=f"ago{step}", tag=f"ago{step}",
                )
                ag_in_r = ag_in[:].rearrange("(t p) c -> p t c", p=128)
                nc.sync.dma_start(out=ag_in_r, in_=g16[:])
                nc.gpsimd.collective_compute(
                    "AllGather",
                    mybir.AluOpType.bypass,
                    replica_groups=rg,
                    ins=[ag_in.opt()],
                    outs=[ag_out.opt()],
                )

                last = step == K
                dsc = d1a_sb if last else d2a_sb
                anchor = a_h0 if last else a_g0
                dst_t = hK if last else g16
                for (t0, t1, c0, c1) in chunks:
                    itile = ip.tile([128, ICH_MAX], i16, tag="it")
                    nc.sync.dma_start(out=itile[:, 0 : c1 - c0], in_=gidx_e[:, c0:c1])
                    for t in range(t0, t1):
                        ws = wsum[t]
                        if ws == 0:
                            continue
                        gt = lp.tile([128, WSUM_MAX, C], fp16, tag="gt")
                        for q in range(NQ):
                            wq = int(W[t, q])
                            if wq == 0:
                                continue
                            a = int(coloff[t, q]) - c0
                            # DMA rings hang above ~1024 descriptors per
                            # gather; split into <=8-column sub-gathers.
                            for o in range(0, wq, 8):
                                w8 = min(8, wq - o)
                                _emit_dma_gather(
                                    nc.gpsimd, mybir, ap_utils,
                                    gt[:, int(qoff[t, q]) + o
                                       : int(qoff[t, q]) + o + w8, :],
                                    ag_out[WINB[q] : WINB[q + 1], 0:C],
                                    itile[:, a + o * 8 : a + (o + w8) * 8],
                                    w8 * 128, C, STRIDE, QMAP[qrr % 8],
                                )
                                qrr += 1
                        agg = mlp.tile([128, C], fp32, tag="agg")
                        if ws == 1:
                            nc.vector.tensor_copy(agg[:], gt[:, 0, :])
                        else:
                            gsub = gt[:, 0:ws, :]
                            gre = bass.AP(
                                gsub.tensor,
                                gsub.offset,
                                [gsub.ap[0], [1, C], [C, ws]],
                            )
                            nc.vector.tensor_reduce(
                                agg[:], gre, axis=mybir.AxisListType.X, op=ALU.add
                            )
                        nc.vector.tensor_tensor(
                            agg[:], agg[:], g16[:, t, 0:C], op=ALU.add
                        )
                        with nc.allow_low_precision(reason="fp16 propagation table"):
                            nc.vector.scalar_tensor_tensor(
                                dst_t[:, t, 0:C],
                                agg[:],
                                dsc[:, t : t + 1],
                                anchor[:, t, :],
                                op0=ALU.mult,
                                op1=ALU.add,
                            )

            # ---- log_softmax over hK
            red = res.tile([128, T, 2], fp32)
            ex = res.tile([128, T, C], fp32)
            nc.vector.tensor_reduce(
                red[:, :, 0:1], hK[:], axis=mybir.AxisListType.X, op=ALU.max
            )
            for t in range(T):
                nc.vector.tensor_scalar(
                    ex[:, t, :], hK[:, t, :], red[:, t, 0:1], None,
                    op0=ALU.subtract,
                )
            nc.scalar.activation(ex[:], ex[:], AF.Exp)
            nc.vector.tensor_reduce(
                red[:, :, 1:2], ex[:], axis=mybir.AxisListType.X, op=ALU.add
            )
            nc.scalar.activation(red[:, :, 1:2], red[:, :, 1:2], AF.Ln)
            for t in range(T):
                nc.vector.tensor_scalar(
                    ex[:, t, :], hK[:, t, :], red[:, t, 0:1], red[:, t, 1:2],
                    op0=ALU.subtract, op1=ALU.subtract,
                )
            out_r = out_e.ap().rearrange("(t p) c -> p t c", p=128)
            nc.sync.dma_start(out=out_r, in_=ex[:])

    nc.finalize()
    return nc


def _run(cfg, x, edge_index, w1, b1, w2, b2, trace=False):
    from concourse import bass_utils

    in_maps, meta = preprocess(cfg, x, edge_index, w1, b1, w2, b2)
    nc = build(cfg, meta)
    res = bass_utils.run_bass_kernel_spmd(
        nc, in_maps, core_ids=list(range(NCORES)), trace=trace
    )
    out = np.empty((cfg.N, C), dtype=np.float32)
    for r in range(NCORES):
        out[meta["ord_g"][r]] = res.results[r]["out"][: cfg.LV]
    return out, res


def kernel(x, edge_index, w1, b1, w2, b2):
    out, _ = _run(
        FULL,
        np.asarray(x),
        np.asarray(edge_index),
        np.asarray(w1),
        np.asarray(b1),
        np.asarray(w2),
        np.asarray(b2),
    )
    return out


# revision 15
# speedup vs baseline: 1.5918x; 1.0698x over previous
"""APPNP (K-step propagation over an MLP) on 8 TRN2 NeuronCores.

Strategy:
  - Nodes are partitioned contiguously across the 8 cores (12500 per core,
    padded to 12544 = 98*128 device-side slots). Within each core, nodes are
    RELABELED in descending in-degree order so fixed-degree padding per
    128-node block is cheap.
  - The dense MLP (lin1/lin2) is data-parallel over nodes.
  - Propagation uses A_hat h = dinv * (A g + g), g = dinv * h, so there is no
    per-edge scaling. Each step: AllGather of the fp16 g shard (padded to
    256B-strided rows) into a full table [NPAD, 128], then the per-edge
    gather runs on the DMAGather Q7 ucode (InstDMAGatherAnt): int16 indices
    limited to 32768 rows force the table into ceil(NPAD/32768) windows; per
    128-node block and window, a fixed number of slot columns W[t][q] is
    gathered with ONE dma_gather each (80B payload per edge, 256B row
    stride). Gathers round-robin over 4 SWDGE queues whose descriptor
    generation runs on dedicated Q7 core pairs in parallel. Per block, the
    vector engine reduces the gathered slots into agg (fp32), adds the self
    term, and a fused scalar_tensor_tensor epilogue forms the next g (fp16).
  - log_softmax at the end on the local shard; host un-permutes rows.
"""

import sys

sys.path.insert(0, "/opt/trn_rl_repo")

import numpy as np

NCORES = 8
ALPHA = 0.1
F_IN = 512
HID = 64
C = 40
STRIDE = 128  # fp16 elems per table row = 256B (dma_gather stride granule)
WINROWS = 32768  # max rows addressable by an int16 gather index
# 4 SWDGE queues (one per Q7 core pair) need the HWDGE ring declarations
# trimmed 16->8, or the collective path runs out of DMA rings and deadlocks.
NQUEUES = 4
IDX_CHUNK_BLOCKS = 8  # blocks per idx-tile DMA


class Cfg:
    def __init__(self, N, LV, L, K):
        assert N == NCORES * LV
        assert L % 128 == 0 and LV < L
        self.N = N
        self.LV = LV
        self.L = L
        self.K = K
        self.T = L // 128
        self.NPAD = NCORES * L


FULL = Cfg(N=100000, LV=12500, L=12544, K=10)


def preprocess(cfg, x, edge_index, w1, b1, w2, b2):
    N, LV, L, T = cfg.N, cfg.LV, cfg.L, cfg.T
    NPAD = cfg.NPAD
    src = np.asarray(edge_index[0], dtype=np.int64)
    dst = np.asarray(edge_index[1], dtype=np.int64)
    E = src.shape[0]

    deg_in = np.bincount(dst, minlength=N)
    deg = (deg_in + 1).astype(np.float32)
    dinv = (1.0 / np.sqrt(deg)).astype(np.float32)

    # per-core relabeling: slot s of core r holds node ord_g[r][s] (global id)
    ord_g = np.empty((NCORES, LV), dtype=np.int64)
    slot_of = np.empty(N, dtype=np.int64)
    for r in range(NCORES):
        ids = np.arange(r * LV, (r + 1) * LV, dtype=np.int64)
        o = ids[np.argsort(-deg_in[ids], kind="stable")]
        ord_g[r] = o
        slot_of[o] = np.arange(LV, dtype=np.int64)

    # table row of a global node in the relabeled padded table
    trow = (np.arange(N) // LV) * L + slot_of

    # index windows (int16 gather index < 32768)
    WINB = list(range(0, NPAD, WINROWS)) + [NPAD]
    NQ = len(WINB) - 1
    # a guaranteed-zero table row inside each window (local index)
    zr_local = []
    pad_rows = [r * L + LV for r in range(NCORES)]
    for q in range(NQ):
        z = next(g for g in pad_rows if WINB[q] <= g < WINB[q + 1])
        zr_local.append(z - WINB[q])

    # per-edge source window + local index
    tsrc = trow[src]
    qe = tsrc // WINROWS
    le = tsrc - qe * WINROWS

    # sort edges by (dst, window); position within each (dst, window) group
    grp = dst * NQ + qe
    order = np.argsort(grp, kind="stable")
    grp_s = grp[order]
    dst_s = dst[order]
    qs_s = qe[order]
    ls_s = le[order]
    cnt = np.bincount(grp_s, minlength=N * NQ)
    starts = np.zeros(N * NQ + 1, dtype=np.int64)
    starts[1:] = np.cumsum(cnt)
    w_s = np.arange(E, dtype=np.int64) - starts[grp_s]
    cnt = cnt.reshape(N, NQ)

    # per-block per-window slot counts (cross-core max)
    W = np.zeros((T, NQ), dtype=np.int64)
    for r in range(NCORES):
        cs = np.zeros((L, NQ), dtype=np.int64)
        cs[:LV] = cnt[ord_g[r]]
        W = np.maximum(W, cs.reshape(T, 128, NQ).max(axis=1))
    wsum = W.sum(axis=1)
    WSUM_MAX = int(max(wsum.max(), 1))
    qoff = np.zeros((T, NQ), dtype=np.int64)
    qoff[:, 1:] = np.cumsum(W, axis=1)[:, :-1]
    icols = wsum * 8  # int16 idx columns per block (128 idx / 16 partitions)
    ioff = np.zeros(T + 1, dtype=np.int64)
    ioff[1:] = np.cumsum(icols)
    ICOLS = int(ioff[-1])
    coloff = ioff[:T, None] + qoff * 8  # [T, NQ]

    # idx-tile DMA chunks (groups of blocks)
    chunks = []
    t0 = 0
    while t0 < T:
        t1 = min(t0 + IDX_CHUNK_BLOCKS, T)
        chunks.append((t0, t1, int(ioff[t0]), int(ioff[t1])))
        t0 = t1
    ICH_MAX = max(c1 - c0 for _, _, c0, c1 in chunks)

    w2p = np.zeros((HID, C), dtype=np.float32)
    w2p[:, :] = np.asarray(w2, dtype=np.float32)
    b2p = np.asarray(b2, dtype=np.float32)
    w1s = np.asarray(w1, dtype=np.float32).reshape(4, 128, HID).transpose(1, 0, 2).copy()
    b1r = np.tile(np.asarray(b1, dtype=np.float32), (128, 1))
    b2r = np.tile(b2p, (128, 1))
    ident = np.eye(128, dtype=np.float32)

    xf = np.asarray(x, dtype=np.float32)
    in_maps = []
    for r in range(NCORES):
        lo, hi = np.searchsorted(dst_s, [r * LV, (r + 1) * LV])
        rows_r = slot_of[dst_s[lo:hi]]
        q_r = qs_s[lo:hi]
        w_r = w_s[lo:hi]
        l_r = ls_s[lo:hi]
        t_r = rows_r // 128
        p_r = rows_r % 128

        idx16 = np.empty((16, ICOLS), dtype=np.int16)
        for t in range(T):
            for q in range(NQ):
                a = coloff[t, q]
                idx16[:, a : a + W[t, q] * 8] = zr_local[q]
        col = coloff[t_r, q_r] + w_r * 8 + p_r // 16
        idx16[p_r % 16, col] = l_r.astype(np.int16)
        idxfull = np.tile(idx16, (8, 1))

        xs = np.zeros((L, F_IN), dtype=np.float32)
        xs[:LV] = xf[ord_g[r]]
        xt = np.ascontiguousarray(xs.T)

        dl = np.zeros((L,), dtype=np.float32)
        dl[:LV] = dinv[ord_g[r]]
        dpt = np.ascontiguousarray(dl.reshape(T, 128).T)

        in_maps.append(
            {
                "xt": xt,
                "w1s": w1s,
                "b1r": b1r,
                "w2p": w2p,
                "b2r": b2r,
                "ident": ident,
                "dinv": dpt,
                "d1a": ((1.0 - ALPHA) * dpt).astype(np.float32),
                "d2a": ((1.0 - ALPHA) * dpt * dpt).astype(np.float32),
                "gidx": idxfull,
            }
        )
    meta = {
        "W": W,
        "qoff": qoff,
        "wsum": [int(v) for v in wsum],
        "WSUM_MAX": WSUM_MAX,
        "coloff": coloff,
        "ioff": [int(v) for v in ioff],
        "ICOLS": ICOLS,
        "ICH_MAX": ICH_MAX,
        "chunks": chunks,
        "WINB": WINB,
        "NQ": NQ,
        "ord_g": ord_g,
    }
    return in_maps, meta


def _emit_dma_gather(gp, mybir, ap_utils, out_ap, in_ap, idxs_ap, num_idxs, elem_size,
                     elem_step, queue_num):
    """bass.dma_gather minus the %256 elem_size assert: the DMAGather ucode in
    non-transpose HBM-source mode only requires the row STRIDE to be a 256B
    multiple; the moved payload (elem_size) is arbitrary."""
    gp._assert_queue_num(queue_num)
    assert idxs_ap.dtype == mybir.dt.int16
    assert in_ap.dtype == out_ap.dtype
    assert ap_utils.ap_is_contiguous(out_ap.ap[1:])
    assert ap_utils.ap_is_contiguous(idxs_ap.ap[1:])
    assert in_ap.ap[-1][1] == out_ap.ap[-1][1] == elem_size
    assert out_ap.ap[0][1] * out_ap.ap[1][1] == ((num_idxs + 127) // 128) * 128
    assert in_ap.ap[0][0] == elem_step
    stride_bytes = elem_step * mybir.dt.size(in_ap.dtype)
    assert stride_bytes % 256 == 0 and 0 < stride_bytes // 256 < 256
    _in_ap = gp.lower_ap_dma(in_ap, for_custom_bir_dma=True)
    _idxs_ap = gp.lower_ap(idxs_ap)
    _out_ap = gp.lower_ap(out_ap)
    return gp.add_instruction(
        mybir.InstDMAGatherAnt(
            name=gp.bass.get_next_instruction_name(),
            ins=[*_in_ap, _idxs_ap, gp.lower_val_access(gp.to_reg(num_idxs))],
            outs=[_out_ap],
            transpose=False,
            num_idxs=num_idxs,
            elem_size=elem_size,
            stride_bytes_256=stride_bytes // 256,
            gen_mode=0,
            single_packet=True,
            queue_num=queue_num,
            sbuf_tokens_per_rank=0,
            sbuf_free_dim_per_rank=0,
            sbuf_free_dim_pad_per_rank=0,
            sbuf_byte_offset=0,
        )
    )


def build(cfg, meta):
    import concourse.bacc as bacc
    import concourse.bass as bass
    import concourse.mybir as mybir
    import concourse.tile as tile
    from concourse import ap_utils, library_config

    fp32 = mybir.dt.float32
    fp16 = mybir.dt.float16
    i16 = mybir.dt.int16
    AF = mybir.ActivationFunctionType
    ALU = mybir.AluOpType

    L, T, K, NPAD = cfg.L, cfg.T, cfg.K, cfg.NPAD
    W, qoff, wsum = meta["W"], meta["qoff"], meta["wsum"]
    WSUM_MAX, coloff, ICOLS = meta["WSUM_MAX"], meta["coloff"], meta["ICOLS"]
    ICH_MAX, chunks, WINB, NQ = meta["ICH_MAX"], meta["chunks"], meta["WINB"], meta["NQ"]

    nc = bacc.Bacc(
        "TRN2", target_bir_lowering=False, debug=False,
        num_devices=NCORES, num_swdge_queues=NQUEUES,
    )
    for _q in nc.m.queues:
        if _q.is_HWDGE:
            _q.num_queues = 8

    xt_e = nc.declare_dram_parameter("xt", [F_IN, L], fp32, isOutput=False)
    w1s_e = nc.declare_dram_parameter("w1s", [128, 4, HID], fp32, isOutput=False)
    b1r_e = nc.declare_dram_parameter("b1r", [128, HID], fp32, isOutput=False)
    w2p_e = nc.declare_dram_parameter("w2p", [HID, C], fp32, isOutput=False)
    b2r_e = nc.declare_dram_parameter("b2r", [128, C], fp32, isOutput=False)
    ident_e = nc.declare_dram_parameter("ident", [128, 128], fp32, isOutput=False)
    dinv_e = nc.declare_dram_parameter("dinv", [128, T], fp32, isOutput=False)
    d1a_e = nc.declare_dram_parameter("d1a", [128, T], fp32, isOutput=False)
    d2a_e = nc.declare_dram_parameter("d2a", [128, T], fp32, isOutput=False)
    gidx_e = nc.declare_dram_parameter("gidx", [128, ICOLS], i16, isOutput=False)
    out_e = nc.declare_dram_parameter("out", [L, C], fp32, isOutput=True)

    with tile.TileContext(nc) as tc:
        with (
            tc.tile_pool(name="res", bufs=1) as res,
            tc.tile_pool(name="dram", bufs=1, space="DRAM") as dram,
            tc.tile_pool(name="mlp", bufs=3) as mlp,
            tc.tile_pool(name="mpsum", bufs=2, space="PSUM") as mpsum,
            tc.tile_pool(name="lp", bufs=3) as lp,
            tc.tile_pool(name="ip", bufs=3) as ip,
        ):
            nc.gpsimd.load_library(library_config.mlp)

            g16 = res.tile([128, T, STRIDE], fp16)
            a_g0 = res.tile([128, T, C], fp32)
            a_h0 = res.tile([128, T, C], fp32)
            hK = res.tile([128, T, C], fp32)
            w1_sb = res.tile([128, 4, HID], fp32)
            b1_sb = res.tile([128, HID], fp32)
            w2_sb = res.tile([HID, C], fp32)
            b2_sb = res.tile([128, C], fp32)
            id_sb = res.tile([128, 128], fp32)
            dinv_sb = res.tile([128, T], fp32)
            d1a_sb = res.tile([128, T], fp32)
            d2a_sb = res.tile([128, T], fp32)

            nc.vector.memset(g16[:], 0.0)
            nc.vector.memset(hK[:], 0.0)
            nc.sync.dma_start(out=w1_sb[:], in_=w1s_e[:, :, :])
            nc.sync.dma_start(out=b1_sb[:], in_=b1r_e[:, :])
            nc.sync.dma_start(out=w2_sb[:], in_=w2p_e[:, :])
            nc.sync.dma_start(out=b2_sb[:], in_=b2r_e[:, :])
            nc.sync.dma_start(out=id_sb[:], in_=ident_e[:, :])
            nc.sync.dma_start(out=dinv_sb[:], in_=dinv_e[:, :])
            nc.sync.dma_start(out=d1a_sb[:], in_=d1a_e[:, :])
            nc.sync.dma_start(out=d2a_sb[:], in_=d2a_e[:, :])

            xt_r = xt_e.ap().rearrange("(kb p) n -> p kb n", p=128)

            # ---- MLP
            for t in range(T):
                xk = mlp.tile([128, 4, 128], fp32, tag="xk")
                nc.sync.dma_start(out=xk[:], in_=xt_r[:, :, t * 128 : (t + 1) * 128])
                ps1 = mpsum.tile([128, HID], fp32, tag="ps1")
                for k in range(4):
                    nc.tensor.matmul(
                        ps1[:], xk[:, k, :], w1_sb[:, k, :],
                        start=(k == 0), stop=(k == 3),
                    )
                h1 = mlp.tile([128, HID], fp32, tag="h1")
                nc.vector.tensor_tensor(h1[:], ps1[:], b1_sb[:], op=ALU.add)
                nc.scalar.activation(h1[:], h1[:], AF.Relu)
                pst = mpsum.tile([128, 128], fp32, tag="pst")
                nc.tensor.transpose(pst[:HID, :], h1[:], id_sb[:])
                h1t = mlp.tile([HID, 128], fp32, tag="h1t")
                nc.vector.tensor_copy(h1t[:], pst[:HID, :])
                ps2 = mpsum.tile([128, C], fp32, tag="ps2")
                nc.tensor.matmul(ps2[:], h1t[:], w2_sb[:], start=True, stop=True)
                h0t = mlp.tile([128, C], fp32, tag="h0t")
                nc.vector.tensor_tensor(h0t[:], ps2[:], b2_sb[:], op=ALU.add)
                with nc.allow_low_precision(reason="fp16 propagation table"):
                    nc.vector.tensor_scalar(
                        g16[:, t, 0:C], h0t[:], dinv_sb[:, t : t + 1], None,
                        op0=ALU.mult,
                    )
                nc.vector.tensor_scalar(
                    a_g0[:, t, :], h0t[:], dinv_sb[:, t : t + 1], ALPHA,
                    op0=ALU.mult, op1=ALU.mult,
                )
                nc.vector.tensor_scalar_mul(a_h0[:, t, :], h0t[:], ALPHA)

            rg = [list(range(NCORES))]
            # The tile framework rotates Pool-engine DMAs over
            # NUM_SWDGE_GLOBAL_SEMS=8 DMASW semaphore lanes in program order,
            # and the runtime locks each lane to one SWDGE queue. Gathers are
            # the only Pool DMAs here, so a fixed lane->queue map keeps every
            # semaphore lane on a single queue while using all 3 queues.
            QMAP = [0, 1, 2, 3, 0, 1, 2, 3]
            qrr = 0  # gather counter (== DMASW lane rotation)
            for step in range(1, K + 1):
                ag_in = dram.tile([L, STRIDE], fp16, name=f"agi{step}", tag=f"agi{step}")
                ag_out = dram.tile(
                    [NPAD, STRIDE], fp16, addr_space="Shared",
                    name=f"ago{step}", tag=f"ago{step}",
                )
                ag_in_r = ag_in[:].rearrange("(t p) c -> p t c", p=128)
                nc.sync.dma_start(out=ag_in_r, in_=g16[:])
                nc.gpsimd.collective_compute(
                    "AllGather",
                    mybir.AluOpType.bypass,
                    replica_groups=rg,
                    ins=[ag_in.opt()],
                    outs=[ag_out.opt()],
                )

                last = step == K
                dsc = d1a_sb if last else d2a_sb
                anchor = a_h0 if last else a_g0
                dst_t = hK if last else g16
                for (t0, t1, c0, c1) in chunks:
                    itile = ip.tile([128, ICH_MAX], i16, tag="it")
                    nc.sync.dma_start(out=itile[:, 0 : c1 - c0], in_=gidx_e[:, c0:c1])
                    for t in range(t0, t1):
                        ws = wsum[t]
                        if ws == 0:
                            continue
                        gt = lp.tile([128, WSUM_MAX, C], fp16, tag="gt")
                        for q in range(NQ):
                            wq = int(W[t, q])
                            if wq == 0:
                                continue
                            a = int(coloff[t, q]) - c0
                            # DMA rings hang above ~1024 descriptors per
                            # gather; split into <=8-column sub-gathers.
                            for o in range(0, wq, 8):
                                w8 = min(8, wq - o)
                                _emit_dma_gather(
                                    nc.gpsimd, mybir, ap_utils,
                                    gt[:, int(qoff[t, q]) + o
                                       : int(qoff[t, q]) + o + w8, :],
                                    ag_out[WINB[q] : WINB[q + 1], 0:C],
                                    itile[:, a + o * 8 : a + (o + w8) * 8],
                                    w8 * 128, C, STRIDE, QMAP[qrr % 8],
                                )
                                qrr += 1
                        # Pairwise tree reduction over the ws gathered slots.
                        # Contiguous slices keep the DVE in fast vector mode
                        # (the strided tensor_reduce ran ~8x slower).
                        facc = mlp.tile(
                            [128, max((WSUM_MAX + 1) // 2, 1), C], fp32, tag="facc"
                        )
                        if ws == 1:
                            nc.vector.tensor_copy(facc[:, 0, :], gt[:, 0, :])
                        else:
                            h = ws // 2
                            m = ws - h  # first-half size (includes odd leftover)
                            nc.vector.tensor_tensor(
                                facc[:, 0:h, :], gt[:, 0:h, :], gt[:, m : ws, :],
                                op=ALU.add,
                            )
                            if m > h:
                                nc.vector.tensor_copy(facc[:, h, :], gt[:, h, :])
                            n = m
                            while n > 1:
                                h = n // 2
                                nc.vector.tensor_tensor(
                                    facc[:, 0:h, :],
                                    facc[:, 0:h, :],
                                    facc[:, n - h : n, :],
                                    op=ALU.add,
                                )
                                n = n - h
                        nc.vector.tensor_tensor(
                            facc[:, 0, :], facc[:, 0, :], g16[:, t, 0:C], op=ALU.add
                        )
                        with nc.allow_low_precision(reason="fp16 propagation table"):
                            nc.vector.scalar_tensor_tensor(
                                dst_t[:, t, 0:C],
                                facc[:, 0, :],
                                dsc[:, t : t + 1],
                                anchor[:, t, :],
                                op0=ALU.mult,
                                op1=ALU.add,
                            )

            # ---- log_softmax over hK
            red = res.tile([128, T, 2], fp32)
            ex = res.tile([128, T, C], fp32)
            nc.vector.tensor_reduce(
                red[:, :, 0:1], hK[:], axis=mybir.AxisListType.X, op=ALU.max
            )
            for t in range(T):
                nc.vector.tensor_scalar(
                    ex[:, t, :], hK[:, t, :], red[:, t, 0:1], None,
                    op0=ALU.subtract,
                )
            nc.scalar.activation(ex[:], ex[:], AF.Exp)
            nc.vector.tensor_reduce(
                red[:, :, 1:2], ex[:], axis=mybir.AxisListType.X, op=ALU.add
            )
            nc.scalar.activation(red[:, :, 1:2], red[:, :, 1:2], AF.Ln)
            for t in range(T):
                nc.vector.tensor_scalar(
                    ex[:, t, :], hK[:, t, :], red[:, t, 0:1], red[:, t, 1:2],
                    op0=ALU.subtract, op1=ALU.subtract,
                )
            out_r = out_e.ap().rearrange("(t p) c -> p t c", p=128)
            nc.sync.dma_start(out=out_r, in_=ex[:])

    nc.finalize()
    return nc


def _run(cfg, x, edge_index, w1, b1, w2, b2, trace=False):
    from concourse import bass_utils

    in_maps, meta = preprocess(cfg, x, edge_index, w1, b1, w2, b2)
    nc = build(cfg, meta)
    res = bass_utils.run_bass_kernel_spmd(
        nc, in_maps, core_ids=list(range(NCORES)), trace=trace
    )
    out = np.empty((cfg.N, C), dtype=np.float32)
    for r in range(NCORES):
        out[meta["ord_g"][r]] = res.results[r]["out"][: cfg.LV]
    return out, res


def kernel(x, edge_index, w1, b1, w2, b2):
    out, _ = _run(
        FULL,
        np.asarray(x),
        np.asarray(edge_index),
        np.asarray(w1),
        np.asarray(b1),
        np.asarray(w2),
        np.asarray(b2),
    )
    return out
